# revision 4
# baseline (speedup 1.0000x reference)
"""Trainium2 Bass kernel for nn_CriticNetwork (gnn_message_passing).

Math: the reference GNN does mean-aggregation over a complete graph with
self-loops, so every node of an env sees the identical per-env mean.  The
whole network collapses to per-env scalars:

  m_b  = mean over the 16 nodes of obs[b]                      [128]
  p_b  = relu(m_b @ W1 + b1) @ W2 + b2                         [64]
  a_b  = p_b . (Wfc @ (Wattn[:64] + Wattn[64:]))               scalar
  w_b  = sigmoid(leaky_relu(a_b, 0.01))                        scalar
  c_b  = p_b . Wv[:64] + bv                                    scalar
  P_bk = pi[b,k] . Wvy ;  A_bk = act[b,k] . Wvy                (Wvy = Wv[64:72])
  xv[b,j] = c_b + (PS_b + w_b*QS_b)/16 - (w_b/16)*(A_bj-P_bj)
  out x[b*16+d, j] = xv[b,j]   (independent of d)
  out w[b*16+d, j] = w_b

Sharding: data-parallel over envs, 512 envs per core x 8 cores.

v2 engine split (v1 ran nearly everything on DVE and serialized outputs):
  - obs mean-trees on DVE, bf16 below the first level
  - per-env MLP chain on PE/ACT in bf16; a [2,257] "replication matmul"
    materializes w*256 | c per env on partitions in one PE op
  - P/A dot products + per-group prefix scalars on GpSimd (otherwise idle)
  - x and w interleave in ONE output tensor xw[row] = [x j0..15 | w j0..15]
    so each group leaves as a single 2KB-per-partition contiguous DMA on the
    sync ring right after the obs stream drains; host splits columns.
"""

import numpy as np
import ml_dtypes
from contextlib import ExitStack

import concourse.bass as bass
import concourse.bacc as bacc
import concourse.tile as tile
from concourse import mybir
from concourse.bass_utils import run_bass_kernel_spmd

B, N, A = 4096, 16, 8
D_IN, H1, DP, DZ = 128, 64, 64, 64
NCORES = 8
BC = B // NCORES          # 512 envs per core
RC = BC * N               # 8192 obs rows per core
G = 4                     # env groups per core (128 envs each)
GE = BC // G              # 128 envs per group

F32 = mybir.dt.float32
BF16 = mybir.dt.bfloat16
ALU = mybir.AluOpType
AFT = mybir.ActivationFunctionType


def _build():
    nc = bacc.Bacc("TRN2", target_bir_lowering=False, debug=False)

    obs = nc.dram_tensor("obs", [RC, D_IN], F32, kind="ExternalInput")
    pol = nc.dram_tensor("pol", [RC, A], F32, kind="ExternalInput")
    act = nc.dram_tensor("act", [RC, A], F32, kind="ExternalInput")
    cst = nc.dram_tensor("cst", [128, 268], F32, kind="ExternalInput")
    cstb = nc.dram_tensor("cstb", [128, 194], BF16, kind="ExternalInput")
    xw = nc.dram_tensor("xw", [RC, 2 * N], F32, kind="ExternalOutput")

    with ExitStack() as ctx:
        tc = ctx.enter_context(tile.TileContext(nc))
        consts = ctx.enter_context(tc.tile_pool(name="consts", bufs=1))
        obsp = ctx.enter_context(tc.tile_pool(name="obsp", bufs=1))
        trp = ctx.enter_context(tc.tile_pool(name="trp", bufs=2))
        pap = ctx.enter_context(tc.tile_pool(name="pap", bufs=1))
        gsp = ctx.enter_context(tc.tile_pool(name="gsp", bufs=1))
        smp = ctx.enter_context(tc.tile_pool(name="smp", bufs=1))
        pmtp = ctx.enter_context(tc.tile_pool(name="pmtp", bufs=2, space="PSUM"))
        php = ctx.enter_context(tc.tile_pool(name="php", bufs=2, space="PSUM"))
        pacp = ctx.enter_context(tc.tile_pool(name="pacp", bufs=2, space="PSUM"))
        pwp = ctx.enter_context(tc.tile_pool(name="pwp", bufs=2, space="PSUM"))

        # ---- input DMAs -------------------------------------------------
        # obs on the sync ring (q1): 8 half-group transfers, issued first.
        # half h of group g = nodes 8h..8h+7 of envs 128g+p (4KB/partition).
        obs_v = obs.ap().rearrange("(g p h nf) f -> g h p (nf f)",
                                   p=128, h=2, nf=8)
        obs_t = []
        for g in range(G):
            pair = []
            for h in range(2):
                t = obsp.tile([128, 8 * 128], F32, name=f"obs{g}{h}")
                nc.sync.dma_start(out=t, in_=obs_v[g][h])
                pair.append(t)
            obs_t.append(pair)

        # consts + pol/act on the scalar ring (q10)
        cst_sb = consts.tile([128, 268], F32)
        nc.scalar.dma_start(out=cst_sb, in_=cst.ap())
        cstb_sb = consts.tile([128, 194], BF16)
        nc.scalar.dma_start(out=cstb_sb, in_=cstb.ap())
        wvy_sb = cst_sb[:, 0:8]
        b1_sb = cst_sb[0:64, 8:9]
        biasq_sb = cst_sb[0:2, 9:10]
        repw_sb = cst_sb[0:2, 10:267]       # [2,257]: row0 -> 0:256, row1 -> 256
        idb_sb = cstb_sb[:, 0:128]
        w1q_sb = cstb_sb[:, 128:192]
        wq_sb = cstb_sb[0:64, 192:194]

        pa_view = lambda t: t.ap().rearrange("(g p n) a -> p g (n a)", p=128, n=16)
        pol_sb = pap.tile([128, G, N * A], F32)
        nc.scalar.dma_start(out=pol_sb, in_=pa_view(pol))
        act_sb = pap.tile([128, G, N * A], F32)
        nc.scalar.dma_start(out=act_sb, in_=pa_view(act))

        # preload the sigmoid ACT table while DMAs stream
        warm = consts.tile([1, 1], F32)
        nc.scalar.activation(out=warm, in_=cst_sb[0:1, 0:1], func=AFT.Sigmoid)

        # per-env scalars, one column pair per group: cols g = w, 4+g = c
        wc8 = smp.tile([128, 8], F32)
        # output: row 16*(128g+p)+d = [x(j=0..15) | w(j=0..15)], 2KB/part/group
        xw_v = xw.ap().rearrange("(g p d) j -> g p (d j)", p=128, d=16)
        xwbig = [smp.tile([128, 16, 2 * N], F32, name=f"xwbig_{g}")
                 for g in range(G)]

        def tree_half(g, h):
            src = obs_t[g][h]
            h1 = trp.tile([128, 512], BF16, name="h1")
            nc.vector.tensor_add(h1, src[:, 0:512], src[:, 512:1024])
            h2 = trp.tile([128, 256], BF16, name="h2")
            nc.vector.tensor_add(h2, h1[:, 0:256], h1[:, 256:512])
            h3 = trp.tile([128, 128], BF16, name=f"h3_{h}")
            nc.vector.tensor_add(h3, h2[:, 0:128], h2[:, 128:256])
            return h3

        def tree_join(g, a3, b3):
            meanS = trp.tile([128, 128], BF16, name="meanS")
            nc.vector.tensor_add(meanS, a3, b3)
            return meanS

        def chain(g, meanS):
            # meanS[p=env, f] --T--> [f, env] -> W1 -> relu -> wq -> [a|c]
            pmt = pmtp.tile([128, 128], BF16, name="pmt")
            nc.tensor.transpose(pmt, meanS[:], idb_sb)
            meanT = trp.tile([128, GE], BF16, name="meanT")
            nc.scalar.activation(out=meanT, in_=pmt, func=AFT.Copy)
            ph = php.tile([64, GE], F32, name="ph")
            nc.tensor.matmul(ph, lhsT=w1q_sb, rhs=meanT[:], start=True, stop=True)
            h_sb = trp.tile([64, GE], BF16, name="h_sb")
            nc.scalar.activation(out=h_sb, in_=ph, func=AFT.Relu, bias=b1_sb)
            pac = pacp.tile([2, GE], F32, name="pac")
            nc.tensor.matmul(pac, lhsT=wq_sb, rhs=h_sb, start=True, stop=True)
            wc = trp.tile([2, GE], F32, name="wc")
            nc.scalar.activation(out=wc, in_=pac, func=AFT.Identity, bias=biasq_sb)
            return wc

        def lr_op(g, wc):
            lr = trp.tile([1, GE], F32, name="lr")
            nc.vector.scalar_tensor_tensor(out=lr, in0=wc[0:1, :], scalar=0.01,
                                           in1=wc[0:1, :], op0=ALU.mult,
                                           op1=ALU.max)
            return lr

        def post(g, wc, lr):
            # w = sigmoid(leaky(a)); [2,128] -> per-env [w*256 | c] via repw,
            # then the w-block of xwbig and the wc8 column pair peel off on ACT
            nc.scalar.activation(out=wc[0:1, :], in_=lr, func=AFT.Sigmoid)
            pw = pwp.tile([128, 257], F32, name="pw")
            nc.tensor.matmul(pw, lhsT=wc[:], rhs=repw_sb, start=True, stop=True)
            nc.scalar.activation(
                out=xwbig[g][:, :, N:2 * N],
                in_=pw[:, 0:256].rearrange("p (d j) -> p d j", j=16),
                func=AFT.Copy)
            wc8_dst = bass.AP(tensor=wc8.tensor, offset=wc8.offset + g,
                              ap=[wc8.ap[0], [4, 2]])
            pw2 = bass.AP(tensor=pw.tensor, offset=pw.offset,
                          ap=[pw.ap[0], [256, 2]])
            nc.scalar.activation(out=wc8_dst, in_=pw2, func=AFT.Copy)

        def combine(g, eng):
            # xwbig[:, :, 0:16] = base + negw*Q  (one fused STT per group)
            # base = (w*QS + PS)/16 + c ; negw = -w/16
            s1 = smp.tile([128, 1], F32, name=f"s1_{g}")
            eng.tensor_tensor(out=s1, in0=QS4[:, g:g + 1], in1=wc8[:, g:g + 1],
                              op=ALU.mult)
            eng.tensor_add(s1, s1, PS4[:, g:g + 1])
            base = smp.tile([128, 1], F32, name=f"base_{g}")
            eng.tensor_scalar_mul(base, s1, 1.0 / N)
            eng.tensor_add(base, base, wc8[:, 4 + g:5 + g])
            negw = smp.tile([128, 1], F32, name=f"negw_{g}")
            eng.tensor_scalar_mul(negw, wc8[:, g:g + 1], -1.0 / N)
            nc.vector.scalar_tensor_tensor(
                out=xwbig[g][:, :, 0:N],
                in0=Q64[:, 16 * g:16 * g + 16].unsqueeze(1)
                    .broadcast_to([128, 16, 16]),
                scalar=negw[:, 0:1],
                in1=base.unsqueeze(1).broadcast_to([128, 16, 16]),
                op0=ALU.mult, op1=ALU.add)
            nc.sync.dma_start(out=xw_v[g],
                              in_=xwbig[g].rearrange("p d j -> p (d j)"))

        # ---- group 0/1 trees, chains -----------------------------------
        a3 = tree_half(0, 0); b3 = tree_half(0, 1); m0 = tree_join(0, a3, b3)
        wc0 = chain(0, m0)
        a3 = tree_half(1, 0); b3 = tree_half(1, 1); m1 = tree_join(1, a3, b3)
        lr0 = lr_op(0, wc0)
        post(0, wc0, lr0)
        wc1 = chain(1, m1)

        # ---- gpsimd P/A block (runs behind pol/act arrival) ------------
        wvyb = wvy_sb.unsqueeze(1).unsqueeze(1).broadcast_to([128, G, 16, 8])
        tmP = gsp.tile([128, G, N * A], F32)
        nc.gpsimd.tensor_tensor(out=tmP.rearrange("p g (r a) -> p g r a", a=8),
                                in0=pol_sb.rearrange("p g (r a) -> p g r a", a=8),
                                in1=wvyb, op=ALU.mult)
        tmA = gsp.tile([128, G, N * A], F32)
        nc.gpsimd.tensor_tensor(out=tmA.rearrange("p g (r a) -> p g r a", a=8),
                                in0=act_sb.rearrange("p g (r a) -> p g r a", a=8),
                                in1=wvyb, op=ALU.mult)

        def gtree8(tm, nm):
            v = tm.rearrange("p g (r a) -> p (g r) a", a=8)
            t1 = gsp.tile([128, 64, 4], F32, name=f"t1{nm}")
            nc.gpsimd.tensor_add(t1, v[:, :, 0:4], v[:, :, 4:8])
            t2 = gsp.tile([128, 64, 2], F32, name=f"t2{nm}")
            nc.gpsimd.tensor_add(t2, t1[:, :, 0:2], t1[:, :, 2:4])
            t3 = gsp.tile([128, 64], F32, name=f"t3{nm}")
            nc.gpsimd.tensor_add(t3, t2[:, :, 0:1].rearrange("p i o -> p (i o)"),
                                 t2[:, :, 1:2].rearrange("p i o -> p (i o)"))
            return t3

        P64 = gtree8(tmP, "P")
        A64 = gtree8(tmA, "A")
        Q64 = gsp.tile([128, 64], F32)
        nc.gpsimd.tensor_sub(Q64, A64, P64)

        def gtree16(t64, nm):
            v = t64.rearrange("p (i n) -> p i n", n=16)
            u1 = gsp.tile([128, 4, 8], F32, name=f"u1{nm}")
            nc.gpsimd.tensor_add(u1, v[:, :, 0:8], v[:, :, 8:16])
            u2 = gsp.tile([128, 4, 4], F32, name=f"u2{nm}")
            nc.gpsimd.tensor_add(u2, u1[:, :, 0:4], u1[:, :, 4:8])
            u3 = gsp.tile([128, 4, 2], F32, name=f"u3{nm}")
            nc.gpsimd.tensor_add(u3, u2[:, :, 0:2], u2[:, :, 2:4])
            u4 = gsp.tile([128, 4], F32, name=f"u4{nm}")
            nc.gpsimd.tensor_add(u4, u3[:, :, 0:1].rearrange("p i o -> p (i o)"),
                                 u3[:, :, 1:2].rearrange("p i o -> p (i o)"))
            return u4

        PS4 = gtree16(P64, "P")
        QS4 = gtree16(Q64, "Q")

        # ---- groups 2/3 interleaved with posts/combines ----------------
        a3 = tree_half(2, 0); b3 = tree_half(2, 1); m2 = tree_join(2, a3, b3)
        lr1 = lr_op(1, wc1)
        post(1, wc1, lr1)
        wc2 = chain(2, m2)
        a33 = tree_half(3, 0)
        lr2 = lr_op(2, wc2)
        post(2, wc2, lr2)
        b33 = tree_half(3, 1)
        m3 = tree_join(3, a33, b33)
        combine(0, nc.gpsimd)
        combine(1, nc.gpsimd)
        combine(2, nc.gpsimd)
        wc3 = chain(3, m3)
        lr3 = lr_op(3, wc3)
        post(3, wc3, lr3)
        combine(3, nc.vector)

    nc.compile()
    return nc


_NC_CACHE = {}


def _get_nc():
    if "nc" not in _NC_CACHE:
        _NC_CACHE["nc"] = _build()
    return _NC_CACHE["nc"]


def _make_in_maps(inputs):
    obs = np.ascontiguousarray(np.asarray(inputs["obs"], np.float32))
    pol = np.ascontiguousarray(np.asarray(inputs["policies"], np.float32))
    act = np.ascontiguousarray(np.asarray(inputs["actions"], np.float32))
    W1 = np.asarray(inputs["W1"], np.float32)
    b1 = np.asarray(inputs["b1"], np.float32)
    W2 = np.asarray(inputs["W2"], np.float32)
    b2 = np.asarray(inputs["b2"], np.float32)
    Wfc = np.asarray(inputs["Wfc"], np.float32)
    Wattn = np.asarray(inputs["Wattn"], np.float32)
    Wv = np.asarray(inputs["Wv"], np.float32)
    bv = np.asarray(inputs["bv"], np.float32)

    wa = (Wfc @ (Wattn[:DZ] + Wattn[DZ:]))[:, 0]     # [64]
    wvy = Wv[DP:, 0]                                  # [8]
    wv64 = Wv[:DP, 0]

    cst = np.zeros((128, 268), np.float32)
    cst[:, 0:8] = wvy[None, :]
    cst[0:64, 8] = b1
    cst[0, 9] = float(b2 @ wa)
    cst[1, 9] = float(b2 @ wv64 + bv[0])
    cst[0, 10:266] = 1.0                     # repw row0: w -> 256 slots
    cst[1, 10:266] = 0.0
    cst[0, 266] = 0.0
    cst[1, 266] = 1.0                        # repw row1: c -> slot 256
    cstb = np.zeros((128, 194), np.float32)
    cstb[:, 0:128] = np.eye(128, dtype=np.float32)
    cstb[:, 128:192] = W1 / 16.0
    cstb[0:64, 192] = W2 @ wa
    cstb[0:64, 193] = W2 @ wv64
    cstb = cstb.astype(ml_dtypes.bfloat16)

    in_maps = []
    for c in range(NCORES):
        in_maps.append({
            "obs": obs[c * RC:(c + 1) * RC],
            "pol": pol[c * RC:(c + 1) * RC],
            "act": act[c * RC:(c + 1) * RC],
            "cst": cst,
            "cstb": cstb,
        })
    return in_maps


# Test-harness knobs (the grader just calls kernel() with defaults).
TRACE = False
TRACE_KWARGS = {}
LAST_RESULT = None


def kernel(**inputs):
    global LAST_RESULT
    nc = _get_nc()
    in_maps = _make_in_maps(inputs)
    res = run_bass_kernel_spmd(nc, in_maps, core_ids=list(range(NCORES)),
                               trace=TRACE, **TRACE_KWARGS)
    LAST_RESULT = res
    xwf = np.concatenate([r["xw"] for r in res.results], axis=0)
    x = np.ascontiguousarray(xwf[:, 0:N]).reshape(B * N, N, 1)
    w = np.ascontiguousarray(xwf[:, N:2 * N]).reshape(B * N, N, 1)
    return x, w


# revision 6
# speedup vs baseline: 1.2108x; 1.2108x over previous
"""Trainium2 Bass kernel for nn_CriticNetwork (gnn_message_passing).

Math: the reference GNN does mean-aggregation over a complete graph with
self-loops, so every node of an env sees the identical per-env mean.  The
whole network collapses to per-env scalars:

  m_b  = mean over the 16 nodes of obs[b]                      [128]
  p_b  = relu(m_b @ W1 + b1) @ W2 + b2                         [64]
  a_b  = p_b . (Wfc @ (Wattn[:64] + Wattn[64:]))               scalar
  w_b  = sigmoid(leaky_relu(a_b, 0.01))                        scalar
  c_b  = p_b . Wv[:64] + bv                                    scalar
  P_bk = pi[b,k] . Wvy ;  A_bk = act[b,k] . Wvy                (Wvy = Wv[64:72])
  xv[b,j] = c_b + (PS_b + w_b*QS_b)/16 - (w_b/16)*(A_bj-P_bj)
  out x[b*16+d, j] = xv[b,j]   (independent of d)
  out w[b*16+d, j] = w_b

Sharding: data-parallel over envs, 512 envs per core x 8 cores.

Scheduling notes (hard-won):
  - the HWDGE ring holds 4 in-flight DMAs per queue: issue N blocks on
    completion of N-4, so the sync ring carries exactly pol, act, obs g0-g2,
    obs g3 in two halves, then the 4 outputs - the first four transfers
    cover enough bytes that later issues are ring-gated before the queue
    ever drains.
  - pol/act lead the obs stream (0.5 MB, ~1.3us) so the GpSimd P/A block
    finishes mid-stream instead of gating the output writes.
  - obs mean-trees on DVE, bf16 below the first level; per-env MLP chain on
    PE/ACT in bf16; a bf16 [2,257] "replication matmul" materializes
    w*256 | c per env on partitions in one cheap PE op.
  - x and w interleave in ONE output tensor xw[row] = [x j0..15 | w j0..15]
    so each group leaves as a single 2KB-per-partition contiguous DMA on
    the sync ring right after the obs stream drains; host splits columns.
"""

import numpy as np
import ml_dtypes
from contextlib import ExitStack

import concourse.bass as bass
import concourse.bacc as bacc
import concourse.tile as tile
from concourse import mybir
from concourse.bass_utils import run_bass_kernel_spmd

B, N, A = 4096, 16, 8
D_IN, H1, DP, DZ = 128, 64, 64, 64
NCORES = 8
BC = B // NCORES          # 512 envs per core
RC = BC * N               # 8192 obs rows per core
G = 4                     # env groups per core (128 envs each)
GE = BC // G              # 128 envs per group

F32 = mybir.dt.float32
BF16 = mybir.dt.bfloat16
ALU = mybir.AluOpType
AFT = mybir.ActivationFunctionType


def _build():
    nc = bacc.Bacc("TRN2", target_bir_lowering=False, debug=False)

    obs = nc.dram_tensor("obs", [RC, D_IN], F32, kind="ExternalInput")
    pol = nc.dram_tensor("pol", [RC, A], F32, kind="ExternalInput")
    act = nc.dram_tensor("act", [RC, A], F32, kind="ExternalInput")
    cst = nc.dram_tensor("cst", [128, 10], F32, kind="ExternalInput")
    cstb = nc.dram_tensor("cstb", [128, 452], BF16, kind="ExternalInput")
    xw = nc.dram_tensor("xw", [RC, 2 * N], F32, kind="ExternalOutput")

    with ExitStack() as ctx:
        tc = ctx.enter_context(tile.TileContext(nc))
        consts = ctx.enter_context(tc.tile_pool(name="consts", bufs=1))
        obsp = ctx.enter_context(tc.tile_pool(name="obsp", bufs=1))
        trp = ctx.enter_context(tc.tile_pool(name="trp", bufs=2))
        pap = ctx.enter_context(tc.tile_pool(name="pap", bufs=1))
        gsp = ctx.enter_context(tc.tile_pool(name="gsp", bufs=1))
        smp = ctx.enter_context(tc.tile_pool(name="smp", bufs=1))
        pmtp = ctx.enter_context(tc.tile_pool(name="pmtp", bufs=2, space="PSUM"))
        php = ctx.enter_context(tc.tile_pool(name="php", bufs=2, space="PSUM"))
        pacp = ctx.enter_context(tc.tile_pool(name="pacp", bufs=2, space="PSUM"))
        pwp = ctx.enter_context(tc.tile_pool(name="pwp", bufs=2, space="PSUM"))

        # ---- input DMAs on the sync ring (q1), in stream order ----------
        pa_view = lambda t: t.ap().rearrange("(g p n) a -> p g (n a)", p=128, n=16)
        pol_sb = pap.tile([128, G, N * A], F32)
        nc.sync.dma_start(out=pol_sb, in_=pa_view(pol))
        act_sb = pap.tile([128, G, N * A], F32)
        nc.sync.dma_start(out=act_sb, in_=pa_view(act))

        # obs: groups 0-2 whole (8KB/partition), group 3 split into node
        # halves so the last tree level can start before the final bytes.
        obs_v = obs.ap().rearrange("(g p nf) f -> g p (nf f)", p=128, nf=16)
        obs_t = []
        for g in range(3):
            t = obsp.tile([128, 16 * 128], F32, name=f"obs{g}")
            nc.sync.dma_start(out=t, in_=obs_v[g])
            obs_t.append(t)
        obs3_v = obs.ap().rearrange("(g p h nf) f -> g h p (nf f)",
                                    p=128, h=2, nf=8)
        obs3 = []
        for h in range(2):
            t = obsp.tile([128, 8 * 128], F32, name=f"obs3{h}")
            nc.sync.dma_start(out=t, in_=obs3_v[3][h])
            obs3.append(t)

        # consts on the scalar ring (q10)
        cst_sb = consts.tile([128, 10], F32)
        nc.scalar.dma_start(out=cst_sb, in_=cst.ap())
        cstb_sb = consts.tile([128, 452], BF16)
        nc.scalar.dma_start(out=cstb_sb, in_=cstb.ap())
        wvy_sb = cst_sb[:, 0:8]
        b1_sb = cst_sb[0:64, 8:9]
        biasq_sb = cst_sb[0:2, 9:10]
        idb_sb = cstb_sb[:, 0:128]
        w1q_sb = cstb_sb[:, 128:192]
        wq_sb = cstb_sb[0:64, 192:194]
        repw_sb = cstb_sb[0:2, 195:452]     # [2,257]: row0 -> 0:256, row1 -> 256

        # preload the sigmoid ACT table while DMAs stream
        warm = consts.tile([1, 1], F32)
        nc.scalar.activation(out=warm, in_=cst_sb[0:1, 0:1], func=AFT.Sigmoid)

        # per-env scalars, one column pair per group: cols g = w, 4+g = c
        wc8 = smp.tile([128, 8], F32)
        # output: row 16*(128g+p)+d = [x(j=0..15) | w(j=0..15)], 2KB/part/group
        xw_v = xw.ap().rearrange("(g p d) j -> g p (d j)", p=128, d=16)
        xwbig = [smp.tile([128, 16, 2 * N], F32, name=f"xwbig_{g}")
                 for g in range(G)]

        def tree_full(g):
            src = obs_t[g]
            s1 = trp.tile([128, 1024], BF16, name="s1")
            nc.vector.tensor_add(s1, src[:, 0:1024], src[:, 1024:2048])
            s2 = trp.tile([128, 512], BF16, name="s2")
            nc.vector.tensor_add(s2, s1[:, 0:512], s1[:, 512:1024])
            s3 = trp.tile([128, 256], BF16, name="s3")
            nc.vector.tensor_add(s3, s2[:, 0:256], s2[:, 256:512])
            meanS = trp.tile([128, 128], BF16, name="meanS")
            nc.vector.tensor_add(meanS, s3[:, 0:128], s3[:, 128:256])
            return meanS

        def tree_half(h):
            src = obs3[h]
            h1 = trp.tile([128, 512], BF16, name="h1")
            nc.vector.tensor_add(h1, src[:, 0:512], src[:, 512:1024])
            h2 = trp.tile([128, 256], BF16, name="h2")
            nc.vector.tensor_add(h2, h1[:, 0:256], h1[:, 256:512])
            h3 = trp.tile([128, 128], BF16, name=f"h3_{h}")
            nc.vector.tensor_add(h3, h2[:, 0:128], h2[:, 128:256])
            return h3

        def chain(g, meanS):
            # meanS[p=env, f] --T--> [f, env] -> W1 -> relu -> wq -> [a|c]
            pmt = pmtp.tile([128, 128], BF16, name="pmt")
            nc.tensor.transpose(pmt, meanS[:], idb_sb)
            meanT = trp.tile([128, GE], BF16, name="meanT")
            nc.scalar.activation(out=meanT, in_=pmt, func=AFT.Copy)
            ph = php.tile([64, GE], F32, name="ph")
            nc.tensor.matmul(ph, lhsT=w1q_sb, rhs=meanT[:], start=True, stop=True)
            h_sb = trp.tile([64, GE], BF16, name="h_sb")
            nc.scalar.activation(out=h_sb, in_=ph, func=AFT.Relu, bias=b1_sb)
            pac = pacp.tile([2, GE], F32, name="pac")
            nc.tensor.matmul(pac, lhsT=wq_sb, rhs=h_sb, start=True, stop=True)
            wcb = trp.tile([2, GE], BF16, name="wcb")
            nc.scalar.activation(out=wcb, in_=pac, func=AFT.Identity, bias=biasq_sb)
            return wcb

        def lr_op(g, wcb):
            lr = trp.tile([1, GE], BF16, name="lr")
            nc.vector.scalar_tensor_tensor(out=lr, in0=wcb[0:1, :], scalar=0.01,
                                           in1=wcb[0:1, :], op0=ALU.mult,
                                           op1=ALU.max)
            return lr

        def post(g, wcb, lr):
            # w = sigmoid(leaky(a)); [2,128] -> per-env [w*256 | c] via repw,
            # then the w-block of xwbig and the wc8 column pair peel off on ACT
            nc.scalar.activation(out=wcb[0:1, :], in_=lr, func=AFT.Sigmoid)
            pw = pwp.tile([128, 257], F32, name="pw")
            nc.tensor.matmul(pw, lhsT=wcb[:], rhs=repw_sb, start=True, stop=True)
            nc.scalar.activation(
                out=xwbig[g][:, :, N:2 * N],
                in_=pw[:, 0:256].rearrange("p (d j) -> p d j", j=16),
                func=AFT.Copy)
            wc8_dst = bass.AP(tensor=wc8.tensor, offset=wc8.offset + g,
                              ap=[wc8.ap[0], [4, 2]])
            pw2 = bass.AP(tensor=pw.tensor, offset=pw.offset,
                          ap=[pw.ap[0], [256, 2]])
            nc.scalar.activation(out=wc8_dst, in_=pw2, func=AFT.Copy)

        def prefix(g, eng):
            # base = (w*QS + PS)/16 + c ; negw = -w/16
            if eng is nc.vector:
                s1 = smp.tile([128, 1], F32, name=f"s1_{g}")
                eng.scalar_tensor_tensor(out=s1, in0=QS4[:, g:g + 1],
                                         scalar=wc8[:, g:g + 1],
                                         in1=PS4[:, g:g + 1],
                                         op0=ALU.mult, op1=ALU.add)
                base = smp.tile([128, 1], F32, name=f"base_{g}")
                eng.scalar_tensor_tensor(out=base, in0=s1, scalar=1.0 / N,
                                         in1=wc8[:, 4 + g:5 + g],
                                         op0=ALU.mult, op1=ALU.add)
            else:
                s1 = smp.tile([128, 1], F32, name=f"s1_{g}")
                eng.tensor_tensor(out=s1, in0=QS4[:, g:g + 1],
                                  in1=wc8[:, g:g + 1], op=ALU.mult)
                eng.tensor_add(s1, s1, PS4[:, g:g + 1])
                base = smp.tile([128, 1], F32, name=f"base_{g}")
                eng.tensor_scalar_mul(base, s1, 1.0 / N)
                eng.tensor_add(base, base, wc8[:, 4 + g:5 + g])
            negw = smp.tile([128, 1], F32, name=f"negw_{g}")
            eng.tensor_scalar_mul(negw, wc8[:, g:g + 1], -1.0 / N)
            return base, negw

        def xcombine(g, base, negw):
            # xwbig[:, :, 0:16] = base + negw*Q  (one fused STT per group)
            nc.vector.scalar_tensor_tensor(
                out=xwbig[g][:, :, 0:N],
                in0=Q64[:, 16 * g:16 * g + 16].unsqueeze(1)
                    .broadcast_to([128, 16, 16]),
                scalar=negw[:, 0:1],
                in1=base.unsqueeze(1).broadcast_to([128, 16, 16]),
                op0=ALU.mult, op1=ALU.add)
            nc.sync.dma_start(out=xw_v[g],
                              in_=xwbig[g].rearrange("p d j -> p (d j)"))

        # ---- group 0/1 trees, chains -----------------------------------
        m0 = tree_full(0)
        wc0 = chain(0, m0)
        m1 = tree_full(1)
        lr0 = lr_op(0, wc0)
        post(0, wc0, lr0)
        wc1 = chain(1, m1)

        # ---- gpsimd P/A block (runs behind pol/act arrival) ------------
        wvyb = wvy_sb.unsqueeze(1).unsqueeze(1).broadcast_to([128, G, 16, 8])
        tmP = gsp.tile([128, G, N * A], F32)
        nc.gpsimd.tensor_tensor(out=tmP.rearrange("p g (r a) -> p g r a", a=8),
                                in0=pol_sb.rearrange("p g (r a) -> p g r a", a=8),
                                in1=wvyb, op=ALU.mult)
        tmA = gsp.tile([128, G, N * A], F32)
        nc.gpsimd.tensor_tensor(out=tmA.rearrange("p g (r a) -> p g r a", a=8),
                                in0=act_sb.rearrange("p g (r a) -> p g r a", a=8),
                                in1=wvyb, op=ALU.mult)

        def gtree8(tm, nm):
            v = tm.rearrange("p g (r a) -> p (g r) a", a=8)
            t1 = gsp.tile([128, 64, 4], F32, name=f"t1{nm}")
            nc.gpsimd.tensor_add(t1, v[:, :, 0:4], v[:, :, 4:8])
            t2 = gsp.tile([128, 64, 2], F32, name=f"t2{nm}")
            nc.gpsimd.tensor_add(t2, t1[:, :, 0:2], t1[:, :, 2:4])
            t3 = gsp.tile([128, 64], F32, name=f"t3{nm}")
            nc.gpsimd.tensor_add(t3, t2[:, :, 0:1].rearrange("p i o -> p (i o)"),
                                 t2[:, :, 1:2].rearrange("p i o -> p (i o)"))
            return t3

        P64 = gtree8(tmP, "P")
        A64 = gtree8(tmA, "A")
        Q64 = gsp.tile([128, 64], F32)
        nc.gpsimd.tensor_sub(Q64, A64, P64)

        def gtree16(t64, nm):
            v = t64.rearrange("p (i n) -> p i n", n=16)
            u1 = gsp.tile([128, 4, 8], F32, name=f"u1{nm}")
            nc.gpsimd.tensor_add(u1, v[:, :, 0:8], v[:, :, 8:16])
            u2 = gsp.tile([128, 4, 4], F32, name=f"u2{nm}")
            nc.gpsimd.tensor_add(u2, u1[:, :, 0:4], u1[:, :, 4:8])
            u3 = gsp.tile([128, 4, 2], F32, name=f"u3{nm}")
            nc.gpsimd.tensor_add(u3, u2[:, :, 0:2], u2[:, :, 2:4])
            u4 = gsp.tile([128, 4], F32, name=f"u4{nm}")
            nc.gpsimd.tensor_add(u4, u3[:, :, 0:1].rearrange("p i o -> p (i o)"),
                                 u3[:, :, 1:2].rearrange("p i o -> p (i o)"))
            return u4

        PS4 = gtree16(P64, "P")
        QS4 = gtree16(Q64, "Q")

        # ---- groups 2/3 interleaved with posts/combines ----------------
        m2 = tree_full(2)
        lr1 = lr_op(1, wc1)
        post(1, wc1, lr1)
        wc2 = chain(2, m2)
        a33 = tree_half(0)
        lr2 = lr_op(2, wc2)
        post(2, wc2, lr2)
        b33 = tree_half(1)
        m3 = trp.tile([128, 128], BF16, name="meanS3")
        nc.vector.tensor_add(m3, a33, b33)
        bn0 = prefix(0, nc.gpsimd)
        bn1 = prefix(1, nc.gpsimd)
        bn2 = prefix(2, nc.gpsimd)
        xcombine(0, *bn0)
        xcombine(1, *bn1)
        xcombine(2, *bn2)
        wc3 = chain(3, m3)
        lr3 = lr_op(3, wc3)
        post(3, wc3, lr3)
        bn3 = prefix(3, nc.vector)
        xcombine(3, *bn3)

    nc.compile()
    return nc


_NC_CACHE = {}


def _get_nc():
    if "nc" not in _NC_CACHE:
        _NC_CACHE["nc"] = _build()
    return _NC_CACHE["nc"]


def _make_in_maps(inputs):
    obs = np.ascontiguousarray(np.asarray(inputs["obs"], np.float32))
    pol = np.ascontiguousarray(np.asarray(inputs["policies"], np.float32))
    act = np.ascontiguousarray(np.asarray(inputs["actions"], np.float32))
    W1 = np.asarray(inputs["W1"], np.float32)
    b1 = np.asarray(inputs["b1"], np.float32)
    W2 = np.asarray(inputs["W2"], np.float32)
    b2 = np.asarray(inputs["b2"], np.float32)
    Wfc = np.asarray(inputs["Wfc"], np.float32)
    Wattn = np.asarray(inputs["Wattn"], np.float32)
    Wv = np.asarray(inputs["Wv"], np.float32)
    bv = np.asarray(inputs["bv"], np.float32)

    wa = (Wfc @ (Wattn[:DZ] + Wattn[DZ:]))[:, 0]     # [64]
    wvy = Wv[DP:, 0]                                  # [8]
    wv64 = Wv[:DP, 0]

    cst = np.zeros((128, 10), np.float32)
    cst[:, 0:8] = wvy[None, :]
    cst[0:64, 8] = b1
    cst[0, 9] = float(b2 @ wa)
    cst[1, 9] = float(b2 @ wv64 + bv[0])
    cstb = np.zeros((128, 452), np.float32)
    cstb[:, 0:128] = np.eye(128, dtype=np.float32)
    cstb[:, 128:192] = W1 / 16.0
    cstb[0:64, 192] = W2 @ wa
    cstb[0:64, 193] = W2 @ wv64
    cstb[0, 195:451] = 1.0                   # repw row0: w -> 256 slots
    cstb[1, 451] = 1.0                       # repw row1: c -> slot 256
    cstb = cstb.astype(ml_dtypes.bfloat16)

    in_maps = []
    for c in range(NCORES):
        in_maps.append({
            "obs": obs[c * RC:(c + 1) * RC],
            "pol": pol[c * RC:(c + 1) * RC],
            "act": act[c * RC:(c + 1) * RC],
            "cst": cst,
            "cstb": cstb,
        })
    return in_maps


# Test-harness knobs (the grader just calls kernel() with defaults).
TRACE = False
TRACE_KWARGS = {}
LAST_RESULT = None


def kernel(**inputs):
    global LAST_RESULT
    nc = _get_nc()
    in_maps = _make_in_maps(inputs)
    res = run_bass_kernel_spmd(nc, in_maps, core_ids=list(range(NCORES)),
                               trace=TRACE, **TRACE_KWARGS)
    LAST_RESULT = res
    xwf = np.concatenate([r["xw"] for r in res.results], axis=0)
    x = np.ascontiguousarray(xwf[:, 0:N]).reshape(B * N, N, 1)
    w = np.ascontiguousarray(xwf[:, N:2 * N]).reshape(B * N, N, 1)
    return x, w


# revision 10
# speedup vs baseline: 1.3257x; 1.0949x over previous
"""Trainium2 Bass kernel for nn_CriticNetwork (gnn_message_passing).

Math: the reference GNN does mean-aggregation over a complete graph with
self-loops, so every node of an env sees the identical per-env mean.  The
whole network collapses to per-env scalars:

  m_b  = mean over the 16 nodes of obs[b]                      [128]
  p_b  = relu(m_b @ W1 + b1) @ W2 + b2                         [64]
  a_b  = p_b . (Wfc @ (Wattn[:64] + Wattn[64:]))               scalar
  w_b  = sigmoid(leaky_relu(a_b, 0.01))                        scalar
  c_b  = p_b . Wv[:64] + bv                                    scalar
  P_bk = pi[b,k] . Wvy ;  A_bk = act[b,k] . Wvy                (Wvy = Wv[64:72])
  xv[b,j] = c_b + (PS_b + w_b*QS_b)/16 - (w_b/16)*(A_bj-P_bj)
  out x[b*16+d, j] = xv[b,j]   (independent of d)
  out w[b*16+d, j] = w_b

Sharding: data-parallel over envs, 512 envs per core x 8 cores.
Within a core, local env e = 4p + s (p = partition, s = group 0..3), so
obs, pol/act and the xw output are all contiguous per partition.

Scheduling notes (hard-won):
  - the HWDGE ring holds 4 in-flight DMAs per queue: issue N blocks on
    completion of N-4.  The sync ring carries pol, act, obs s0-s2, obs s3
    in two node-halves, then the 4 outputs; the first four transfers cover
    enough bytes that later issues ring-gate before the queue drains.
  - pol/act lead the obs stream as single fully-contiguous 2KB/partition
    transfers so the GpSimd P/A block finishes mid-stream.
  - obs mean-trees on DVE, bf16 below the first level; per-env MLP chain
    on PE/ACT in bf16; a bf16 [2,257] "replication matmul" materializes
    w*256 | c per env on partitions in one cheap PE op.  Group 3's two
    tree halves transpose-accumulate straight into PSUM, skipping the join.
  - the tile scheduler reorders per-engine streams; add_dep_helper pins
    the small leaky/combine DVE ops behind the tree levels so they never
    head-of-line-block the stream-paced trees.
  - x and w interleave in ONE output tensor xw[row] = [x j0..15 | w j0..15]
    so each group leaves as a single 2KB-per-partition contiguous DMA.
"""

import numpy as np
import ml_dtypes
from contextlib import ExitStack

import concourse.bass as bass
import concourse.bacc as bacc
import concourse.tile as tile
from concourse import mybir
from concourse.bass_utils import run_bass_kernel_spmd

B, N, A = 4096, 16, 8
D_IN, H1, DP, DZ = 128, 64, 64, 64
NCORES = 8
BC = B // NCORES          # 512 envs per core
RC = BC * N               # 8192 obs rows per core
G = 4                     # env groups per core (128 envs each)
GE = BC // G              # 128 envs per group

F32 = mybir.dt.float32
BF16 = mybir.dt.bfloat16
ALU = mybir.AluOpType
AFT = mybir.ActivationFunctionType


def _dep(later, earlier):
    if later is not None and earlier is not None:
        tile.add_dep_helper(later.ins, earlier.ins, sync=False,
                            reason="engine stream order")


def _build():
    nc = bacc.Bacc("TRN2", target_bir_lowering=False, debug=False)

    obs = nc.dram_tensor("obs", [RC, D_IN], F32, kind="ExternalInput")
    pol = nc.dram_tensor("pol", [RC, A], F32, kind="ExternalInput")
    act = nc.dram_tensor("act", [RC, A], F32, kind="ExternalInput")
    cst = nc.dram_tensor("cst", [128, 10], F32, kind="ExternalInput")
    cstb = nc.dram_tensor("cstb", [128, 452], BF16, kind="ExternalInput")
    xw = nc.dram_tensor("xw", [RC, 2 * N], F32, kind="ExternalOutput")

    with ExitStack() as ctx:
        tc = ctx.enter_context(tile.TileContext(nc))
        consts = ctx.enter_context(tc.tile_pool(name="consts", bufs=1))
        obsp = ctx.enter_context(tc.tile_pool(name="obsp", bufs=1))
        trp = ctx.enter_context(tc.tile_pool(name="trp", bufs=2))
        pap = ctx.enter_context(tc.tile_pool(name="pap", bufs=1))
        gsp = ctx.enter_context(tc.tile_pool(name="gsp", bufs=1))
        smp = ctx.enter_context(tc.tile_pool(name="smp", bufs=1))
        pmtp = ctx.enter_context(tc.tile_pool(name="pmtp", bufs=2, space="PSUM"))
        php = ctx.enter_context(tc.tile_pool(name="php", bufs=2, space="PSUM"))
        pacp = ctx.enter_context(tc.tile_pool(name="pacp", bufs=1, space="PSUM"))
        pwp = ctx.enter_context(tc.tile_pool(name="pwp", bufs=2, space="PSUM"))

        # ---- input DMAs on the sync ring (q1), in stream order ----------
        # pol/act fully contiguous: partition p = rows 64p..64p+63
        pol_sb = pap.tile([128, 512], F32)
        nc.sync.dma_start(out=pol_sb,
                          in_=pol.ap().rearrange("(p r) a -> p (r a)", p=128))
        act_sb = pap.tile([128, 512], F32)
        nc.sync.dma_start(out=act_sb,
                          in_=act.ap().rearrange("(p r) a -> p (r a)", p=128))

        # obs: groups 0-2 whole (8KB/partition contiguous), group 3 in two
        # node halves so the last tree level starts before the final bytes.
        obs_v = obs.ap().rearrange("(p s n) f -> s p (n f)", p=128, n=16)
        obs_t = []
        for g in range(3):
            t = obsp.tile([128, 16 * 128], F32, name=f"obs{g}")
            nc.sync.dma_start(out=t, in_=obs_v[g])
            obs_t.append(t)
        obs3_v = obs.ap().rearrange("(p s h nf) f -> s h p (nf f)",
                                    p=128, h=2, nf=8)
        obs3 = []
        for h in range(2):
            t = obsp.tile([128, 8 * 128], F32, name=f"obs3{h}")
            nc.sync.dma_start(out=t, in_=obs3_v[3][h])
            obs3.append(t)

        # consts on the scalar ring (q10)
        cst_sb = consts.tile([128, 10], F32)
        nc.scalar.dma_start(out=cst_sb, in_=cst.ap())
        cstb_sb = consts.tile([128, 452], BF16)
        nc.scalar.dma_start(out=cstb_sb, in_=cstb.ap())
        wvy_sb = cst_sb[:, 0:8]
        b1_sb = cst_sb[0:64, 8:9]
        biasq_sb = cst_sb[0:2, 9:10]
        idb_sb = cstb_sb[:, 0:128]
        w1q_sb = cstb_sb[:, 128:192]
        wq_sb = cstb_sb[0:64, 192:194]
        repw_sb = cstb_sb[0:2, 195:452]     # [2,257]: row0 -> 0:256, row1 -> 256

        # preload the sigmoid ACT table while DMAs stream
        warm = consts.tile([1, 1], F32)
        nc.scalar.activation(out=warm, in_=cst_sb[0:1, 0:1], func=AFT.Sigmoid)

        # per-env scalars, one column pair per group: cols g = w, 4+g = c
        wc8 = smp.tile([128, 8], F32)
        # output: row 64p+16s+d = [x(j=0..15) | w(j=0..15)], 2KB/part/group
        xw_v = xw.ap().rearrange("(p s d) j -> s p (d j)", p=128, d=16)
        xwbig = [smp.tile([128, 16, 2 * N], F32, name=f"xwbig_{g}")
                 for g in range(G)]

        def tree_full(g):
            src = obs_t[g]
            s1 = trp.tile([128, 1024], BF16, name="s1")
            nc.vector.tensor_add(s1, src[:, 0:1024], src[:, 1024:2048])
            s2 = trp.tile([128, 512], BF16, name="s2")
            nc.vector.tensor_add(s2, s1[:, 0:512], s1[:, 512:1024])
            s3 = trp.tile([128, 256], BF16, name="s3")
            nc.vector.tensor_add(s3, s2[:, 0:256], s2[:, 256:512])
            meanS = trp.tile([128, 128], BF16, name="meanS")
            last = nc.vector.tensor_add(meanS, s3[:, 0:128], s3[:, 128:256])
            return meanS, last

        def tree_half(h):
            src = obs3[h]
            h1 = trp.tile([128, 512], BF16, name="h1")
            nc.vector.tensor_add(h1, src[:, 0:512], src[:, 512:1024])
            h2 = trp.tile([128, 256], BF16, name="h2")
            nc.vector.tensor_add(h2, h1[:, 0:256], h1[:, 256:512])
            h3 = trp.tile([128, 128], BF16, name=f"h3_{h}")
            last = nc.vector.tensor_add(h3, h2[:, 0:128], h2[:, 128:256])
            return h3, last

        def chain_from_pmt(g, pmt):
            meanT = trp.tile([128, GE], BF16, name="meanT")
            nc.scalar.activation(out=meanT, in_=pmt, func=AFT.Copy)
            ph = php.tile([64, GE], F32, name="ph")
            nc.tensor.matmul(ph, lhsT=w1q_sb, rhs=meanT[:], start=True, stop=True)
            h_sb = trp.tile([64, GE], BF16, name="h_sb")
            nc.scalar.activation(out=h_sb, in_=ph, func=AFT.Relu, bias=b1_sb)
            pac = pacp.tile([2, GE], F32, name="pac")
            nc.tensor.matmul(pac, lhsT=wq_sb, rhs=h_sb, start=True, stop=True)
            wcb = trp.tile([2, GE], BF16, name="wcb")
            nc.scalar.activation(out=wcb, in_=pac, func=AFT.Identity, bias=biasq_sb)
            return wcb

        def chain(g, meanS):
            pmt = pmtp.tile([128, 128], BF16, name="pmt")
            nc.tensor.transpose(pmt, meanS[:], idb_sb)
            return chain_from_pmt(g, pmt)

        def lr_op(g, wcb):
            lr = trp.tile([1, GE], BF16, name="lr")
            i = nc.vector.scalar_tensor_tensor(out=lr, in0=wcb[0:1, :], scalar=0.01,
                                               in1=wcb[0:1, :], op0=ALU.mult,
                                               op1=ALU.max)
            return lr, i

        def post(g, wcb, lr):
            # w = sigmoid(leaky(a)); [2,128] -> per-env [w*256 | c] via repw.
            # wc8 column pair peels off first (it gates prefix/combine);
            # the fat w-block copy runs off the critical path.
            nc.scalar.activation(out=wcb[0:1, :], in_=lr, func=AFT.Sigmoid)
            pw = pwp.tile([128, 257], F32, name="pw")
            nc.tensor.matmul(pw, lhsT=wcb[:], rhs=repw_sb, start=True, stop=True)
            wc8_dst = bass.AP(tensor=wc8.tensor, offset=wc8.offset + g,
                              ap=[wc8.ap[0], [4, 2]])
            pw2 = bass.AP(tensor=pw.tensor, offset=pw.offset,
                          ap=[pw.ap[0], [256, 2]])
            nc.scalar.activation(out=wc8_dst, in_=pw2, func=AFT.Copy)
            nc.scalar.activation(
                out=xwbig[g][:, :, N:2 * N],
                in_=pw[:, 0:256].rearrange("p (d j) -> p d j", j=16),
                func=AFT.Copy)

        def prefix(g, eng):
            # base = (w*QS + PS)/16 + c ; negw = -w/16
            if eng is nc.vector:
                s1 = smp.tile([128, 1], F32, name=f"s1_{g}")
                eng.scalar_tensor_tensor(out=s1, in0=QS4[:, g:g + 1],
                                         scalar=wc8[:, g:g + 1],
                                         in1=PS4[:, g:g + 1],
                                         op0=ALU.mult, op1=ALU.add)
                base = smp.tile([128, 1], F32, name=f"base_{g}")
                eng.scalar_tensor_tensor(out=base, in0=s1, scalar=1.0 / N,
                                         in1=wc8[:, 4 + g:5 + g],
                                         op0=ALU.mult, op1=ALU.add)
            else:
                s1 = smp.tile([128, 1], F32, name=f"s1_{g}")
                eng.tensor_tensor(out=s1, in0=QS4[:, g:g + 1],
                                  in1=wc8[:, g:g + 1], op=ALU.mult)
                eng.tensor_add(s1, s1, PS4[:, g:g + 1])
                base = smp.tile([128, 1], F32, name=f"base_{g}")
                eng.tensor_scalar_mul(base, s1, 1.0 / N)
                eng.tensor_add(base, base, wc8[:, 4 + g:5 + g])
            negw = smp.tile([128, 1], F32, name=f"negw_{g}")
            eng.tensor_scalar_mul(negw, wc8[:, g:g + 1], -1.0 / N)
            return base, negw

        def xcombine(g, base, negw):
            # xwbig[:, :, 0:16] = base + negw*Q  (one fused STT per group)
            i = nc.vector.scalar_tensor_tensor(
                out=xwbig[g][:, :, 0:N],
                in0=Q64[:, 16 * g:16 * g + 16].unsqueeze(1)
                    .broadcast_to([128, 16, 16]),
                scalar=negw[:, 0:1],
                in1=base.unsqueeze(1).broadcast_to([128, 16, 16]),
                op0=ALU.mult, op1=ALU.add)
            nc.sync.dma_start(out=xw_v[g],
                              in_=xwbig[g].rearrange("p d j -> p (d j)"))
            return i

        # ---- group 0/1 trees, chains -----------------------------------
        m0, t0l = tree_full(0)
        wc0 = chain(0, m0)
        m1, t1l = tree_full(1)
        lr0, lr0i = lr_op(0, wc0)
        _dep(lr0i, t1l)              # lr never blocks a stream-paced tree
        post(0, wc0, lr0)
        wc1 = chain(1, m1)

        # ---- gpsimd P/A block (runs behind pol/act arrival) ------------
        pol4 = pol_sb.rearrange("p (s r a) -> p s r a", s=G, a=8)
        act4 = act_sb.rearrange("p (s r a) -> p s r a", s=G, a=8)
        wvyb = wvy_sb.unsqueeze(1).unsqueeze(1).broadcast_to([128, G, 16, 8])
        tmP = gsp.tile([128, G, N * A], F32)
        nc.gpsimd.tensor_tensor(out=tmP.rearrange("p g (r a) -> p g r a", a=8),
                                in0=pol4, in1=wvyb, op=ALU.mult)
        tmA = gsp.tile([128, G, N * A], F32)
        nc.gpsimd.tensor_tensor(out=tmA.rearrange("p g (r a) -> p g r a", a=8),
                                in0=act4, in1=wvyb, op=ALU.mult)

        def gtree8(tm, nm):
            v = tm.rearrange("p g (r a) -> p (g r) a", a=8)
            t1 = gsp.tile([128, 64, 4], F32, name=f"t1{nm}")
            nc.gpsimd.tensor_add(t1, v[:, :, 0:4], v[:, :, 4:8])
            t2 = gsp.tile([128, 64, 2], F32, name=f"t2{nm}")
            nc.gpsimd.tensor_add(t2, t1[:, :, 0:2], t1[:, :, 2:4])
            t3 = gsp.tile([128, 64], F32, name=f"t3{nm}")
            nc.gpsimd.tensor_add(t3, t2[:, :, 0:1].rearrange("p i o -> p (i o)"),
                                 t2[:, :, 1:2].rearrange("p i o -> p (i o)"))
            return t3

        P64 = gtree8(tmP, "P")
        A64 = gtree8(tmA, "A")
        Q64 = gsp.tile([128, 64], F32)
        nc.gpsimd.tensor_sub(Q64, A64, P64)

        def gtree16(t64, nm):
            v = t64.rearrange("p (i n) -> p i n", n=16)
            u1 = gsp.tile([128, 4, 8], F32, name=f"u1{nm}")
            nc.gpsimd.tensor_add(u1, v[:, :, 0:8], v[:, :, 8:16])
            u2 = gsp.tile([128, 4, 4], F32, name=f"u2{nm}")
            nc.gpsimd.tensor_add(u2, u1[:, :, 0:4], u1[:, :, 4:8])
            u3 = gsp.tile([128, 4, 2], F32, name=f"u3{nm}")
            nc.gpsimd.tensor_add(u3, u2[:, :, 0:2], u2[:, :, 2:4])
            u4 = gsp.tile([128, 4], F32, name=f"u4{nm}")
            nc.gpsimd.tensor_add(u4, u3[:, :, 0:1].rearrange("p i o -> p (i o)"),
                                 u3[:, :, 1:2].rearrange("p i o -> p (i o)"))
            return u4

        PS4 = gtree16(P64, "P")
        QS4 = gtree16(Q64, "Q")

        # ---- groups 2/3 interleaved with posts/combines ----------------
        m2, t2l = tree_full(2)
        lr1, lr1i = lr_op(1, wc1)
        _dep(lr1i, t2l)
        post(1, wc1, lr1)
        wc2 = chain(2, m2)
        a33, a33l = tree_half(0)
        lr2, lr2i = lr_op(2, wc2)
        _dep(lr2i, a33l)
        post(2, wc2, lr2)
        b33, b33l = tree_half(1)
        m3 = trp.tile([128, 128], BF16, name="meanS3")
        b33l = nc.vector.tensor_add(m3, a33, b33)
        pmt3 = pmtp.tile([128, 128], BF16, name="pmt3", bufs=1)
        nc.tensor.transpose(pmt3, m3[:], idb_sb)
        bn0 = prefix(0, nc.gpsimd)
        bn1 = prefix(1, nc.gpsimd)
        bn2 = prefix(2, nc.gpsimd)
        x0i = xcombine(0, *bn0)
        _dep(x0i, b33l)              # combines stay behind the last tree
        x1i = xcombine(1, *bn1)
        _dep(x1i, b33l)
        x2i = xcombine(2, *bn2)
        _dep(x2i, b33l)
        wc3 = chain_from_pmt(3, pmt3)
        lr3, lr3i = lr_op(3, wc3)
        _dep(lr3i, b33l)
        post(3, wc3, lr3)
        bn3 = prefix(3, nc.vector)
        xcombine(3, *bn3)

    nc.compile()
    return nc


_NC_CACHE = {}


def _get_nc():
    if "nc" not in _NC_CACHE:
        _NC_CACHE["nc"] = _build()
    return _NC_CACHE["nc"]


def _make_in_maps(inputs):
    obs = np.ascontiguousarray(np.asarray(inputs["obs"], np.float32))
    pol = np.ascontiguousarray(np.asarray(inputs["policies"], np.float32))
    act = np.ascontiguousarray(np.asarray(inputs["actions"], np.float32))
    W1 = np.asarray(inputs["W1"], np.float32)
    b1 = np.asarray(inputs["b1"], np.float32)
    W2 = np.asarray(inputs["W2"], np.float32)
    b2 = np.asarray(inputs["b2"], np.float32)
    Wfc = np.asarray(inputs["Wfc"], np.float32)
    Wattn = np.asarray(inputs["Wattn"], np.float32)
    Wv = np.asarray(inputs["Wv"], np.float32)
    bv = np.asarray(inputs["bv"], np.float32)

    wa = (Wfc @ (Wattn[:DZ] + Wattn[DZ:]))[:, 0]     # [64]
    wvy = Wv[DP:, 0]                                  # [8]
    wv64 = Wv[:DP, 0]

    cst = np.zeros((128, 10), np.float32)
    cst[:, 0:8] = wvy[None, :]
    cst[0:64, 8] = b1
    cst[0, 9] = float(b2 @ wa)
    cst[1, 9] = float(b2 @ wv64 + bv[0])
    cstb = np.zeros((128, 452), np.float32)
    cstb[:, 0:128] = np.eye(128, dtype=np.float32)
    cstb[:, 128:192] = W1 / 16.0
    cstb[0:64, 192] = W2 @ wa
    cstb[0:64, 193] = W2 @ wv64
    cstb[0, 195:451] = 1.0                   # repw row0: w -> 256 slots
    cstb[1, 451] = 1.0                       # repw row1: c -> slot 256
    cstb = cstb.astype(ml_dtypes.bfloat16)

    in_maps = []
    for c in range(NCORES):
        in_maps.append({
            "obs": obs[c * RC:(c + 1) * RC],
            "pol": pol[c * RC:(c + 1) * RC],
            "act": act[c * RC:(c + 1) * RC],
            "cst": cst,
            "cstb": cstb,
        })
    return in_maps


# Test-harness knobs (the grader just calls kernel() with defaults).
TRACE = False
TRACE_KWARGS = {}
LAST_RESULT = None


def kernel(**inputs):
    global LAST_RESULT
    nc = _get_nc()
    in_maps = _make_in_maps(inputs)
    res = run_bass_kernel_spmd(nc, in_maps, core_ids=list(range(NCORES)),
                               trace=TRACE, **TRACE_KWARGS)
    LAST_RESULT = res
    # local env e = 4p+s; rows (p, s, d) enumerate (e, d) in order, so the
    # concatenated [RC, 32] block is already (global row, [x16 | w16]).
    xwf = np.concatenate([r["xw"] for r in res.results], axis=0)
    x = np.ascontiguousarray(xwf[:, 0:N]).reshape(B * N, N, 1)
    w = np.ascontiguousarray(xwf[:, N:2 * N]).reshape(B * N, N, 1)
    return x, w


# revision 14
# speedup vs baseline: 1.3496x; 1.0180x over previous
"""Trainium2 Bass kernel for nn_CriticNetwork (gnn_message_passing).

Math: the reference GNN does mean-aggregation over a complete graph with
self-loops, so every node of an env sees the identical per-env mean.  The
whole network collapses to per-env scalars:

  m_b  = mean over the 16 nodes of obs[b]                      [128]
  p_b  = relu(m_b @ W1 + b1) @ W2 + b2                         [64]
  a_b  = p_b . (Wfc @ (Wattn[:64] + Wattn[64:]))               scalar
  w_b  = sigmoid(leaky_relu(a_b, 0.01))                        scalar
  c_b  = p_b . Wv[:64] + bv                                    scalar
  P_bk = pi[b,k] . Wvy ;  A_bk = act[b,k] . Wvy                (Wvy = Wv[64:72])
  xv[b,j] = c_b + (PS_b + w_b*QS_b)/16 - (w_b/16)*(A_bj-P_bj)
  out x[b*16+d, j] = xv[b,j]   (independent of d)
  out w[b*16+d, j] = w_b

Sharding: data-parallel over envs, 512 envs per core x 8 cores.
Within a core, local env e = 4p + s (p = partition, s = group 0..3), so
obs, pol/act and the xw output are all contiguous per partition.

Scheduling notes (hard-won):
  - the HWDGE ring holds 4 in-flight DMAs per queue: issue N blocks on
    completion of N-4.  The sync ring carries pol, act, obs s0-s2, obs s3
    in two node-halves, then the 4 outputs; the first four transfers cover
    enough bytes that later issues ring-gate before the queue drains.
  - pol/act lead the obs stream as single fully-contiguous 2KB/partition
    transfers so the GpSimd P/A block finishes mid-stream.
  - obs mean-trees on DVE, bf16 below the first level; per-env MLP chain
    on PE/ACT in bf16; a bf16 [2,257] "replication matmul" materializes
    w*256 | c per env on partitions in one cheap PE op.  Group 3's two
    tree halves transpose-accumulate straight into PSUM, skipping the join.
  - the tile scheduler reorders per-engine streams; add_dep_helper pins
    the small leaky/combine DVE ops behind the tree levels so they never
    head-of-line-block the stream-paced trees.
  - x and w interleave in ONE output tensor xw[row] = [x j0..15 | w j0..15]
    so each group leaves as a single 2KB-per-partition contiguous DMA.
"""

import numpy as np
import ml_dtypes
from contextlib import ExitStack

import concourse.bass as bass
import concourse.bacc as bacc
import concourse.tile as tile
from concourse import mybir
from concourse.bass_utils import run_bass_kernel_spmd

B, N, A = 4096, 16, 8
D_IN, H1, DP, DZ = 128, 64, 64, 64
NCORES = 8
BC = B // NCORES          # 512 envs per core
RC = BC * N               # 8192 obs rows per core
G = 4                     # env groups per core (128 envs each)
GE = BC // G              # 128 envs per group

F32 = mybir.dt.float32
BF16 = mybir.dt.bfloat16
ALU = mybir.AluOpType
AFT = mybir.ActivationFunctionType


def _dep(later, earlier):
    if later is not None and earlier is not None:
        tile.add_dep_helper(later.ins, earlier.ins, sync=False,
                            reason="engine stream order")


def _build():
    nc = bacc.Bacc("TRN2", target_bir_lowering=False, debug=False)

    obs = nc.dram_tensor("obs", [RC, D_IN], F32, kind="ExternalInput")
    pol = nc.dram_tensor("pol", [RC, A], F32, kind="ExternalInput")
    act = nc.dram_tensor("act", [RC, A], F32, kind="ExternalInput")
    cst = nc.dram_tensor("cst", [128, 10], F32, kind="ExternalInput")
    cstb = nc.dram_tensor("cstb", [128, 452], BF16, kind="ExternalInput")
    xw = nc.dram_tensor("xw", [RC, 2 * N], F32, kind="ExternalOutput")

    with ExitStack() as ctx:
        tc = ctx.enter_context(tile.TileContext(nc))
        consts = ctx.enter_context(tc.tile_pool(name="consts", bufs=1))
        obsp = ctx.enter_context(tc.tile_pool(name="obsp", bufs=1))
        trp = ctx.enter_context(tc.tile_pool(name="trp", bufs=2))
        pap = ctx.enter_context(tc.tile_pool(name="pap", bufs=1))
        gsp = ctx.enter_context(tc.tile_pool(name="gsp", bufs=1))
        smp = ctx.enter_context(tc.tile_pool(name="smp", bufs=1))
        pmtp = ctx.enter_context(tc.tile_pool(name="pmtp", bufs=2, space="PSUM"))
        php = ctx.enter_context(tc.tile_pool(name="php", bufs=2, space="PSUM"))
        pacp = ctx.enter_context(tc.tile_pool(name="pacp", bufs=1, space="PSUM"))
        pwp = ctx.enter_context(tc.tile_pool(name="pwp", bufs=2, space="PSUM"))

        # ---- input DMAs on the sync ring (q1), in stream order ----------
        # obs0 leads (a 1MB transfer saturates the DMA engines immediately
        # and nothing upstream of tree0 is needed earlier); pol/act ride
        # behind it as single fully-contiguous 2KB/partition transfers.
        obs_v = obs.ap().rearrange("(p s n) f -> s p (n f)", p=128, n=16)
        obs_t = []
        t = obsp.tile([128, 16 * 128], F32, name="obs0")
        nc.sync.dma_start(out=t, in_=obs_v[0])
        obs_t.append(t)
        pol_sb = pap.tile([128, 512], F32)
        nc.sync.dma_start(out=pol_sb,
                          in_=pol.ap().rearrange("(p r) a -> p (r a)", p=128))
        act_sb = pap.tile([128, 512], F32)
        nc.sync.dma_start(out=act_sb,
                          in_=act.ap().rearrange("(p r) a -> p (r a)", p=128))

        # obs groups 1-2 whole (8KB/partition contiguous), group 3 in two
        # node halves so the last tree level starts before the final bytes.
        for g in range(1, 3):
            t = obsp.tile([128, 16 * 128], F32, name=f"obs{g}")
            nc.sync.dma_start(out=t, in_=obs_v[g])
            obs_t.append(t)
        obs3_v = obs.ap().rearrange("(p s h nf) f -> s h p (nf f)",
                                    p=128, h=2, nf=8)
        obs3 = []
        for h in range(2):
            t = obsp.tile([128, 8 * 128], F32, name=f"obs3{h}")
            nc.sync.dma_start(out=t, in_=obs3_v[3][h])
            obs3.append(t)

        # consts on the scalar ring (q10)
        cst_sb = consts.tile([128, 10], F32)
        nc.scalar.dma_start(out=cst_sb, in_=cst.ap())
        cstb_sb = consts.tile([128, 452], BF16)
        nc.scalar.dma_start(out=cstb_sb, in_=cstb.ap())
        wvy_sb = cst_sb[:, 0:8]
        b1_sb = cst_sb[0:64, 8:9]
        biasq_sb = cst_sb[0:2, 9:10]
        idb_sb = cstb_sb[:, 0:128]
        w1q_sb = cstb_sb[:, 128:192]
        wq_sb = cstb_sb[0:64, 192:194]
        repw_sb = cstb_sb[0:2, 195:452]     # [2,257]: row0 -> 0:256, row1 -> 256

        # preload the sigmoid ACT table while DMAs stream
        warm = consts.tile([1, 1], F32)
        nc.scalar.activation(out=warm, in_=cst_sb[0:1, 0:1], func=AFT.Sigmoid)

        # per-env scalars, one column pair per group: cols g = w, 4+g = c
        wc8 = smp.tile([128, 8], F32)
        # output: row 64p+16s+d = [x(j=0..15) | w(j=0..15)], 2KB/part/group
        xw_v = xw.ap().rearrange("(p s d) j -> s p (d j)", p=128, d=16)
        xwbig = [smp.tile([128, 16, 2 * N], F32, name=f"xwbig_{g}")
                 for g in range(G)]

        def tree_full(g):
            src = obs_t[g]
            s1 = trp.tile([128, 1024], BF16, name="s1")
            nc.vector.tensor_add(s1, src[:, 0:1024], src[:, 1024:2048])
            s2 = trp.tile([128, 512], BF16, name="s2")
            nc.vector.tensor_add(s2, s1[:, 0:512], s1[:, 512:1024])
            s3 = trp.tile([128, 256], BF16, name="s3")
            nc.vector.tensor_add(s3, s2[:, 0:256], s2[:, 256:512])
            meanS = trp.tile([128, 128], BF16, name="meanS")
            last = nc.vector.tensor_add(meanS, s3[:, 0:128], s3[:, 128:256])
            return meanS, last

        def tree_half(h):
            src = obs3[h]
            h1 = trp.tile([128, 512], BF16, name="h1")
            nc.vector.tensor_add(h1, src[:, 0:512], src[:, 512:1024])
            h2 = trp.tile([128, 256], BF16, name="h2")
            nc.vector.tensor_add(h2, h1[:, 0:256], h1[:, 256:512])
            h3 = trp.tile([128, 128], BF16, name=f"h3_{h}")
            last = nc.vector.tensor_add(h3, h2[:, 0:128], h2[:, 128:256])
            return h3, last

        def chain_from_pmt(g, pmt):
            meanT = trp.tile([128, GE], BF16, name="meanT")
            nc.scalar.activation(out=meanT, in_=pmt, func=AFT.Copy)
            ph = php.tile([64, GE], F32, name="ph")
            nc.tensor.matmul(ph, lhsT=w1q_sb, rhs=meanT[:], start=True, stop=True)
            h_sb = trp.tile([64, GE], BF16, name="h_sb")
            nc.scalar.activation(out=h_sb, in_=ph, func=AFT.Relu, bias=b1_sb)
            pac = pacp.tile([2, GE], F32, name="pac")
            nc.tensor.matmul(pac, lhsT=wq_sb, rhs=h_sb, start=True, stop=True)
            wcb = trp.tile([2, GE], BF16, name="wcb")
            nc.scalar.activation(out=wcb, in_=pac, func=AFT.Identity, bias=biasq_sb)
            return wcb

        def chain(g, meanS):
            pmt = pmtp.tile([128, 128], BF16, name="pmt")
            nc.tensor.transpose(pmt, meanS[:], idb_sb)
            return chain_from_pmt(g, pmt)

        def lr_op(g, wcb):
            lr = trp.tile([1, GE], BF16, name="lr")
            i = nc.vector.scalar_tensor_tensor(out=lr, in0=wcb[0:1, :], scalar=0.01,
                                               in1=wcb[0:1, :], op0=ALU.mult,
                                               op1=ALU.max)
            return lr, i

        def post(g, wcb, lr):
            # w = sigmoid(leaky(a)); [2,128] -> per-env [w*256 | c] via repw.
            # For groups 0-2 the wc8 column pair peels off first (it gates
            # the gpsimd prefix); group 3's prefix reads the PSUM directly.
            nc.scalar.activation(out=wcb[0:1, :], in_=lr, func=AFT.Sigmoid)
            pw = pwp.tile([128, 257], F32, name="pw")
            nc.tensor.matmul(pw, lhsT=wcb[:], rhs=repw_sb, start=True, stop=True)
            if g < 3:
                wc8_dst = bass.AP(tensor=wc8.tensor, offset=wc8.offset + g,
                                  ap=[wc8.ap[0], [4, 2]])
                pw2 = bass.AP(tensor=pw.tensor, offset=pw.offset,
                              ap=[pw.ap[0], [256, 2]])
                nc.scalar.activation(out=wc8_dst, in_=pw2, func=AFT.Copy)
            nc.scalar.activation(
                out=xwbig[g][:, :, N:2 * N],
                in_=pw[:, 0:256].rearrange("p (d j) -> p d j", j=16),
                func=AFT.Copy)
            return pw

        def prefix(g, eng, wcol=None, ccol=None):
            # base = (w*QS + PS)/16 + c ; negw = -w/16
            wcol = wc8[:, g:g + 1] if wcol is None else wcol
            ccol = wc8[:, 4 + g:5 + g] if ccol is None else ccol
            if eng is nc.vector:
                s1 = smp.tile([128, 1], F32, name=f"s1_{g}")
                eng.scalar_tensor_tensor(out=s1, in0=QS4[:, g:g + 1],
                                         scalar=wcol, in1=PS4[:, g:g + 1],
                                         op0=ALU.mult, op1=ALU.add)
                base = smp.tile([128, 1], F32, name=f"base_{g}")
                eng.scalar_tensor_tensor(out=base, in0=s1, scalar=1.0 / N,
                                         in1=ccol, op0=ALU.mult, op1=ALU.add)
            else:
                s1 = smp.tile([128, 1], F32, name=f"s1_{g}")
                eng.tensor_tensor(out=s1, in0=QS4[:, g:g + 1],
                                  in1=wcol, op=ALU.mult)
                eng.tensor_add(s1, s1, PS4[:, g:g + 1])
                base = smp.tile([128, 1], F32, name=f"base_{g}")
                eng.tensor_scalar_mul(base, s1, 1.0 / N)
                eng.tensor_add(base, base, ccol)
            negw = smp.tile([128, 1], F32, name=f"negw_{g}")
            eng.tensor_scalar_mul(negw, wcol, -1.0 / N)
            return base, negw

        def xcombine(g, base, negw):
            # xwbig[:, :, 0:16] = base + negw*Q  (one fused STT per group)
            i = nc.vector.scalar_tensor_tensor(
                out=xwbig[g][:, :, 0:N],
                in0=Q64[:, 16 * g:16 * g + 16].unsqueeze(1)
                    .broadcast_to([128, 16, 16]),
                scalar=negw[:, 0:1],
                in1=base.unsqueeze(1).broadcast_to([128, 16, 16]),
                op0=ALU.mult, op1=ALU.add)
            nc.sync.dma_start(out=xw_v[g],
                              in_=xwbig[g].rearrange("p d j -> p (d j)"))
            return i

        # ---- group 0/1 trees, chains -----------------------------------
        m0, t0l = tree_full(0)
        wc0 = chain(0, m0)
        m1, t1l = tree_full(1)
        lr0, lr0i = lr_op(0, wc0)
        _dep(lr0i, t1l)              # lr never blocks a stream-paced tree
        post(0, wc0, lr0)
        wc1 = chain(1, m1)

        # ---- gpsimd P/A block (runs behind pol/act arrival) ------------
        pol4 = pol_sb.rearrange("p (s r a) -> p s r a", s=G, a=8)
        act4 = act_sb.rearrange("p (s r a) -> p s r a", s=G, a=8)
        wvyb = wvy_sb.unsqueeze(1).unsqueeze(1).broadcast_to([128, G, 16, 8])
        tmP = gsp.tile([128, G, N * A], F32)
        nc.gpsimd.tensor_tensor(out=tmP.rearrange("p g (r a) -> p g r a", a=8),
                                in0=pol4, in1=wvyb, op=ALU.mult)
        tmA = gsp.tile([128, G, N * A], F32)
        nc.gpsimd.tensor_tensor(out=tmA.rearrange("p g (r a) -> p g r a", a=8),
                                in0=act4, in1=wvyb, op=ALU.mult)

        def gtree8(tm, nm):
            v = tm.rearrange("p g (r a) -> p (g r) a", a=8)
            t1 = gsp.tile([128, 64, 4], F32, name=f"t1{nm}")
            nc.gpsimd.tensor_add(t1, v[:, :, 0:4], v[:, :, 4:8])
            t2 = gsp.tile([128, 64, 2], F32, name=f"t2{nm}")
            nc.gpsimd.tensor_add(t2, t1[:, :, 0:2], t1[:, :, 2:4])
            t3 = gsp.tile([128, 64], F32, name=f"t3{nm}")
            nc.gpsimd.tensor_add(t3, t2[:, :, 0:1].rearrange("p i o -> p (i o)"),
                                 t2[:, :, 1:2].rearrange("p i o -> p (i o)"))
            return t3

        P64 = gtree8(tmP, "P")
        A64 = gtree8(tmA, "A")
        Q64 = gsp.tile([128, 64], F32)
        nc.gpsimd.tensor_sub(Q64, A64, P64)

        def gtree16(t64, nm):
            v = t64.rearrange("p (i n) -> p i n", n=16)
            u1 = gsp.tile([128, 4, 8], F32, name=f"u1{nm}")
            nc.gpsimd.tensor_add(u1, v[:, :, 0:8], v[:, :, 8:16])
            u2 = gsp.tile([128, 4, 4], F32, name=f"u2{nm}")
            nc.gpsimd.tensor_add(u2, u1[:, :, 0:4], u1[:, :, 4:8])
            u3 = gsp.tile([128, 4, 2], F32, name=f"u3{nm}")
            nc.gpsimd.tensor_add(u3, u2[:, :, 0:2], u2[:, :, 2:4])
            u4 = gsp.tile([128, 4], F32, name=f"u4{nm}")
            nc.gpsimd.tensor_add(u4, u3[:, :, 0:1].rearrange("p i o -> p (i o)"),
                                 u3[:, :, 1:2].rearrange("p i o -> p (i o)"))
            return u4

        PS4 = gtree16(P64, "P")
        QS4 = gtree16(Q64, "Q")

        # ---- groups 2/3 interleaved with posts/combines ----------------
        m2, t2l = tree_full(2)
        lr1, lr1i = lr_op(1, wc1)
        _dep(lr1i, t2l)
        post(1, wc1, lr1)
        wc2 = chain(2, m2)
        a33, a33l = tree_half(0)
        lr2, lr2i = lr_op(2, wc2)
        _dep(lr2i, a33l)
        post(2, wc2, lr2)
        b33, b33l = tree_half(1)
        m3 = trp.tile([128, 128], BF16, name="meanS3")
        b33l = nc.vector.tensor_add(m3, a33, b33)
        pmt3 = pmtp.tile([128, 128], BF16, name="pmt3", bufs=1)
        nc.tensor.transpose(pmt3, m3[:], idb_sb)
        bn0 = prefix(0, nc.gpsimd)
        bn1 = prefix(1, nc.gpsimd)
        bn2 = prefix(2, nc.gpsimd)
        x0i = xcombine(0, *bn0)
        _dep(x0i, b33l)              # combines stay behind the last tree
        x1i = xcombine(1, *bn1)
        _dep(x1i, b33l)
        x2i = xcombine(2, *bn2)
        _dep(x2i, b33l)
        wc3 = chain_from_pmt(3, pmt3)
        lr3, lr3i = lr_op(3, wc3)
        _dep(lr3i, b33l)
        pw3 = post(3, wc3, lr3)
        bn3 = prefix(3, nc.vector, wcol=pw3[:, 0:1], ccol=pw3[:, 256:257])
        xcombine(3, *bn3)

    nc.compile()
    return nc


_NC_CACHE = {}


def _get_nc():
    if "nc" not in _NC_CACHE:
        _NC_CACHE["nc"] = _build()
    return _NC_CACHE["nc"]


def _make_in_maps(inputs):
    obs = np.ascontiguousarray(np.asarray(inputs["obs"], np.float32))
    pol = np.ascontiguousarray(np.asarray(inputs["policies"], np.float32))
    act = np.ascontiguousarray(np.asarray(inputs["actions"], np.float32))
    W1 = np.asarray(inputs["W1"], np.float32)
    b1 = np.asarray(inputs["b1"], np.float32)
    W2 = np.asarray(inputs["W2"], np.float32)
    b2 = np.asarray(inputs["b2"], np.float32)
    Wfc = np.asarray(inputs["Wfc"], np.float32)
    Wattn = np.asarray(inputs["Wattn"], np.float32)
    Wv = np.asarray(inputs["Wv"], np.float32)
    bv = np.asarray(inputs["bv"], np.float32)

    wa = (Wfc @ (Wattn[:DZ] + Wattn[DZ:]))[:, 0]     # [64]
    wvy = Wv[DP:, 0]                                  # [8]
    wv64 = Wv[:DP, 0]

    cst = np.zeros((128, 10), np.float32)
    cst[:, 0:8] = wvy[None, :]
    cst[0:64, 8] = b1
    cst[0, 9] = float(b2 @ wa)
    cst[1, 9] = float(b2 @ wv64 + bv[0])
    cstb = np.zeros((128, 452), np.float32)
    cstb[:, 0:128] = np.eye(128, dtype=np.float32)
    cstb[:, 128:192] = W1 / 16.0
    cstb[0:64, 192] = W2 @ wa
    cstb[0:64, 193] = W2 @ wv64
    cstb[0, 195:451] = 1.0                   # repw row0: w -> 256 slots
    cstb[1, 451] = 1.0                       # repw row1: c -> slot 256
    cstb = cstb.astype(ml_dtypes.bfloat16)

    in_maps = []
    for c in range(NCORES):
        in_maps.append({
            "obs": obs[c * RC:(c + 1) * RC],
            "pol": pol[c * RC:(c + 1) * RC],
            "act": act[c * RC:(c + 1) * RC],
            "cst": cst,
            "cstb": cstb,
        })
    return in_maps


# Test-harness knobs (the grader just calls kernel() with defaults).
TRACE = False
TRACE_KWARGS = {}
LAST_RESULT = None


def kernel(**inputs):
    global LAST_RESULT
    nc = _get_nc()
    in_maps = _make_in_maps(inputs)
    res = run_bass_kernel_spmd(nc, in_maps, core_ids=list(range(NCORES)),
                               trace=TRACE, **TRACE_KWARGS)
    LAST_RESULT = res
    # local env e = 4p+s; rows (p, s, d) enumerate (e, d) in order, so the
    # concatenated [RC, 32] block is already (global row, [x16 | w16]).
    xwf = np.concatenate([r["xw"] for r in res.results], axis=0)
    x = np.ascontiguousarray(xwf[:, 0:N]).reshape(B * N, N, 1)
    w = np.ascontiguousarray(xwf[:, N:2 * N]).reshape(B * N, N, 1)
    return x, w
